# revision 14
# baseline (speedup 1.0000x reference)
"""Trainium2 Bass kernel for nn_AttentionEdgeDecoder.

Reference computation (per batch b):
  hn = h[b,:4096,:], hg = h[b,4096,:]
  q = hg @ W_q  (single query, 8 heads x 16 dims)
  k,v = hn @ W_kv ; attn = softmax(q.k/sqrt(16)) ; y = attn.v
  mh = y @ W_mhc ; y2[i] = <mh, hn[i]>             (4096 scalars)
  e[i,j] = y2[j]*W_lin[0,0] + y2[i]*W_lin[1,0]     (4096x4096 output)

Output is 4*4096^2*4B = 268MB -> HBM-write bound. Sharding: 8 cores =
4 batches x 2 row-halves; each core computes y2[b] redundantly (tiny) and
streams its (2048, 4096) block of e to DRAM at the per-core HBM limit.

TensorEngine formulation (out = lhsT.T @ rhs, all f32):
  q_col   = matmul(lhsT=W_q, rhs=hg_col)                  [128,1]
  Qh      = headmask * q_col   (block-diag scatter)       [128,8]
  Wqeff   = matmul(lhsT=WkT, rhs=Qh) = Wk @ Qh            [128,8]
  sT      = matmul(lhsT=hnT_chunk, rhs=Wqeff)             [4096,8] j-partitioned
  pT      = exp(0.25*sT)      (no max-subtract: |s/4| < 8)
  u_raw   = sum_chunks matmul(lhsT=pT_chunk, rhs=hn_chunk)   [8,128]
  ssum    = matmul(lhsT=ones, rhs=pT) -> strided reduce      [1,8]
  u       = u_raw * (1/ssum)                                 [8,128]
  ymatT   = matmul(lhsT=Wv, rhs=u^T)                         [128,8]
  y_col   = reduce_h(ymatT * headmask)                       [128,1]
  mh_row  = matmul(lhsT=y_col, rhs=W_mhc)                    [1,128]
  mh0_rep = (W0*mh_row) x ones  (outer, K=1)                 [128,128]
  R       = matmul(lhsT=mh0_rep, rhs=hnT) = bcast W0*y2[j]   [128,4096]
  mh1_col = matmul(lhsT=mh_row, rhs=W1)  (K=1)               [128,1]
  col     = matmul(lhsT=hrT_tile, rhs=mh1_col) = W1*y2[rows] [128,16]
  e_tile  = tensor_scalar_add(R, col[:,t])  -> 4MB DMAs out

sT/exp/u are pipelined per 4-chunk group (PE alternates sT and u while the
Scalar engine does the group's exp). A burst of dummy identity matmuls at
t=0 warms the PE (HAM clock gate 4/8 -> 8/8) while input DMAs stream on
both HWDGE rings (sync + scalar).
"""

from contextlib import ExitStack

import numpy as np

import concourse.bass as bass
import concourse.mybir as mybir
from concourse import bacc, tile
from concourse.bass_utils import run_bass_kernel_spmd

BP = 4
N = 4096
HID = 128
H = 8
D = 16
ROWS = N // 2          # 2048 rows per core
NT = ROWS // 128       # 16 row tiles per core
NJC = N // 128         # 32 node chunks
GRP = 4                # node chunks per exp/u pipeline group
F32 = mybir.dt.float32

TPC = 2                # row-tiles per output chunk (4MB DMAs)
NCHUNK = NT // TPC
NWARM = 20             # PE warm-up matmuls (~4.3us cold -> HAM 8/8)

# wpack column layout
WKT0 = 0               # WkT = W_kv[:, :128].T
WV0 = HID              # Wv  = W_kv[:, 128:]
WMHC0 = 2 * HID
WQ0 = 3 * HID
ID0 = 4 * HID          # 128x128 identity
MSK0 = 5 * HID         # head mask [128, 8]
HG0 = 5 * HID + H      # hg column
WL0 = HG0 + 1          # W_lin row (partition 0)
WPACK_COLS = WL0 + 2


def build_bass():
    nc = bacc.Bacc()

    wpack_ext = nc.declare_dram_parameter("wpack", [HID, WPACK_COLS], F32, isOutput=False)
    hnT_ext = nc.declare_dram_parameter("hnT", [HID, N], F32, isOutput=False)
    # hnp: hn pre-packed on host to [p, jc, c] = hn[jc*128+p, c], contiguous
    hnp_ext = nc.declare_dram_parameter("hnp", [128, NJC * HID], F32, isOutput=False)
    hrT_ext = nc.declare_dram_parameter("hrT", [HID, ROWS], F32, isOutput=False)
    out_ext = nc.declare_dram_parameter("out", [ROWS, N], F32, isOutput=True)

    with tile.TileContext(nc) as tc, ExitStack() as ctx:
        sb = ctx.enter_context(tc.tile_pool(name="sb", bufs=1))
        small = ctx.enter_context(tc.tile_pool(name="small", bufs=1))
        ps_acc = ctx.enter_context(tc.tile_pool(name="ps_acc", bufs=1, space="PSUM"))
        ps_tmp = ctx.enter_context(tc.tile_pool(name="ps_tmp", bufs=2, space="PSUM"))
        ps_big = ctx.enter_context(tc.tile_pool(name="ps_big", bufs=3, space="PSUM"))
        epool = ctx.enter_context(tc.tile_pool(name="epool", bufs=3))

        # ---- input DMAs: sync ring takes wpack+hnT, scalar ring hnp+hrT ----
        wpack_sb = sb.tile([HID, WPACK_COLS], F32)
        nc.sync.dma_start(wpack_sb[:], wpack_ext[:, :])
        hnT_sb = sb.tile([HID, N], F32)
        for k in range(8):
            nc.sync.dma_start(
                hnT_sb[:, bass.ts(k, N // 8)], hnT_ext[:, bass.ts(k, N // 8)]
            )
        hn_sb = sb.tile([128, NJC, HID], F32)
        hn_flat = hn_sb[:].rearrange("p a b -> p (a b)")
        for k in range(8):
            nc.scalar.dma_start(
                hn_flat[:, bass.ts(k, NJC * HID // 8)],
                hnp_ext[:, bass.ts(k, NJC * HID // 8)],
            )
        hrT_sb = sb.tile([HID, ROWS], F32)
        for k in range(2):
            nc.scalar.dma_start(
                hrT_sb[:, bass.ts(k, ROWS // 2)], hrT_ext[:, bass.ts(k, ROWS // 2)]
            )

        identity = wpack_sb[:, ID0:ID0 + HID]
        mask_ap = wpack_sb[:, MSK0:MSK0 + H]

        # ---- PE warm-up: dense dependency-free matmuls right after wpack ----
        for w in range(NWARM):
            warm_ps = ps_big.tile([128, 128], F32, tag="big")
            nc.tensor.matmul(warm_ps[:], identity, identity, start=True, stop=True)

        ones_row = small.tile([1, 128], F32)
        nc.vector.memset(ones_row[:], 1.0)
        ones_col = small.tile([128, 1], F32)
        nc.vector.memset(ones_col[:], 1.0)

        # ---- attention prologue ----
        # q_col = W_q.T @ hg
        q_ps = ps_tmp.tile([HID, 1], F32, tag="tmp")
        nc.tensor.matmul(
            q_ps[:], wpack_sb[:, WQ0:WQ0 + HID], wpack_sb[:, HG0:HG0 + 1],
            start=True, stop=True,
        )
        q_sb = small.tile([HID, 1], F32)
        nc.scalar.copy(q_sb[:], q_ps[:])

        # Qh block-diag scatter: Qh[e, h] = mask[e, h] * q[e]
        qh_sb = small.tile([HID, H], F32)
        nc.vector.tensor_scalar_mul(qh_sb[:], mask_ap, q_sb[:])

        # Wqeff = Wk @ Qh
        wqeff_ps = ps_tmp.tile([HID, H], F32, tag="tmp")
        nc.tensor.matmul(
            wqeff_ps[:], wpack_sb[:, WKT0:WKT0 + HID], qh_sb[:], start=True, stop=True
        )
        wqeff_sb = small.tile([HID, H], F32)
        nc.scalar.copy(wqeff_sb[:], wqeff_ps[:])

        # sT -> exp -> u, pipelined per 4-chunk group
        sT_ps = ps_acc.tile([128, NJC, H], F32)
        pT_sb = small.tile([128, NJC, H], F32)
        u_ps = ps_acc.tile([H, HID], F32)
        ngrp = NJC // GRP
        for g in range(ngrp):
            for jc in range(g * GRP, (g + 1) * GRP):
                nc.tensor.matmul(
                    sT_ps[:, jc, :],
                    hnT_sb[:, bass.ts(jc, 128)],
                    wqeff_sb[:],
                    start=True,
                    stop=True,
                )
            nc.scalar.activation(
                pT_sb[:, bass.ts(g, GRP), :],
                sT_ps[:, bass.ts(g, GRP), :],
                mybir.ActivationFunctionType.Exp,
                scale=0.25,
            )
            for jc in range(g * GRP, (g + 1) * GRP):
                nc.tensor.matmul(
                    u_ps[:],
                    pT_sb[:, jc, :],
                    hn_sb[:, jc, :],
                    start=(jc == 0),
                    stop=(jc == NJC - 1),
                    skip_group_check=True,
                )

        # ssum[h] = sum_j pT[j, h] via ones-matmul, then reduce over chunks
        sums_ps = ps_tmp.tile([1, NJC * H], F32, tag="tmp")
        nc.tensor.matmul(
            sums_ps[:],
            ones_col[:],
            pT_sb[:].rearrange("p a b -> p (a b)"),
            start=True,
            stop=True,
        )
        ssum_sb = small.tile([1, H], F32)
        nc.vector.tensor_reduce(
            ssum_sb[:],
            sums_ps[:].rearrange("p (a b) -> p b a", b=H),
            axis=mybir.AxisListType.X,
            op=mybir.AluOpType.add,
        )
        rr_sb = small.tile([1, H], F32)
        nc.vector.reciprocal(rr_sb[:], ssum_sb[:])
        rr_ps = ps_tmp.tile([H, 1], F32, tag="tmp")
        nc.tensor.transpose(rr_ps[:], rr_sb[:], identity[0:1, 0:1])
        rs_sb = small.tile([H, 1], F32)
        nc.scalar.copy(rs_sb[:], rr_ps[:])

        # u = u_raw / ssum  (per-partition scalar multiply)
        u_sb = small.tile([H, HID], F32)
        nc.vector.tensor_scalar_mul(u_sb[:], u_ps[:], rs_sb[:])

        # uT
        uT_ps = ps_tmp.tile([HID, H], F32, tag="tmp")
        nc.tensor.transpose(uT_ps[:], u_sb[:], identity[0:H, 0:H])
        uT_sb = small.tile([HID, H], F32)
        nc.scalar.copy(uT_sb[:], uT_ps[:])

        # ymatT = Wv.T @ uT  -> [e, h]
        ymatT_ps = ps_tmp.tile([HID, H], F32, tag="tmp")
        nc.tensor.matmul(
            ymatT_ps[:], wpack_sb[:, WV0:WV0 + HID], uT_sb[:], start=True, stop=True
        )
        # y_col[e] = ymatT[e, head(e)] = sum_h ymatT[e, h] * mask[e, h]
        ymm_sb = small.tile([HID, H], F32)
        y_sb = small.tile([HID, 1], F32)
        nc.vector.tensor_mul(ymm_sb[:], ymatT_ps[:], mask_ap)
        nc.vector.tensor_reduce(
            y_sb[:], ymm_sb[:], axis=mybir.AxisListType.X, op=mybir.AluOpType.add
        )

        # mh_row = y.T @ W_mhc
        mh_ps = ps_tmp.tile([1, HID], F32, tag="tmp")
        nc.tensor.matmul(
            mh_ps[:], y_sb[:], wpack_sb[:, WMHC0:WMHC0 + HID], start=True, stop=True
        )
        mh_sb = small.tile([1, HID], F32)
        nc.scalar.copy(mh_sb[:], mh_ps[:])

        # mh0_row = W0 * mh_row ; mh0_rep[c, p] = mh0[c] (128 identical cols)
        mh0_sb = small.tile([1, HID], F32)
        nc.vector.tensor_scalar_mul(mh0_sb[:], mh_sb[:], wpack_sb[0:1, WL0:WL0 + 1])
        mh0rep_ps = ps_tmp.tile([HID, HID], F32, tag="tmp")
        nc.tensor.matmul(mh0rep_ps[:], mh0_sb[:], ones_row[:], start=True, stop=True)
        mh0rep_sb = small.tile([HID, HID], F32)
        nc.scalar.copy(mh0rep_sb[:], mh0rep_ps[:])

        # mh1_col[c] = mh[c] * W1  (K=1 outer product with scalar)
        mh1_ps = ps_tmp.tile([HID, 1], F32, tag="tmp")
        nc.tensor.matmul(
            mh1_ps[:], mh_sb[:], wpack_sb[0:1, WL0 + 1:WL0 + 2], start=True, stop=True
        )
        mh1_sb = small.tile([HID, 1], F32)
        nc.scalar.copy(mh1_sb[:], mh1_ps[:])

        # R[p, j] = W0*y2[j]: lhsT = mh0_rep (stationary), rhs = hnT chunks
        r_sb = sb.tile([128, N], F32)
        for k in range(8):
            rb_ps = ps_big.tile([128, 512], F32, tag="big")
            nc.tensor.matmul(
                rb_ps[:], mh0rep_sb[:], hnT_sb[:, bass.ts(k, 512)],
                start=True, stop=True,
            )
            if k % 2 == 0:
                nc.vector.tensor_copy(r_sb[:, bass.ts(k, 512)], rb_ps[:])
            else:
                nc.scalar.copy(r_sb[:, bass.ts(k, 512)], rb_ps[:])

        # col[p, t] = W1*y2[r0 + t*128 + p]
        col_ps = ps_acc.tile([128, NT], F32)
        for t in range(NT):
            nc.tensor.matmul(
                col_ps[:, t : t + 1],
                hrT_sb[:, bass.ts(t, 128)],
                mh1_sb[:],
                start=True,
                stop=True,
            )
        col_sb = small.tile([128, NT], F32)
        nc.vector.tensor_copy(col_sb[:], col_ps[:])

        # ---- epilogue: e tiles + DMA out ----
        out_r = out_ext[:, :].rearrange("(o s p) j -> o p s j", s=TPC, p=128)
        for ot in range(NCHUNK):
            etile = epool.tile([128, TPC, N], F32)
            for s in range(TPC):
                t = ot * TPC + s
                nc.vector.tensor_scalar_add(
                    etile[:, s, :], r_sb[:], col_sb[:, t : t + 1]
                )
            nc.sync.dma_start(out_r[ot], etile[:])

    nc.finalize()
    return nc


_CACHED = {}


def _get_nc():
    if "nc" not in _CACHED:
        _CACHED["nc"] = build_bass()
    return _CACHED["nc"]


def _make_wpack(W_q, W_kv, W_mhc, W_lin):
    wpack = np.zeros((HID, WPACK_COLS), dtype=np.float32)
    wpack[:, WKT0:WKT0 + HID] = W_kv[:, :HID].T
    wpack[:, WV0:WV0 + HID] = W_kv[:, HID:]
    wpack[:, WMHC0:WMHC0 + HID] = W_mhc
    wpack[:, WQ0:WQ0 + HID] = W_q
    wpack[:, ID0:ID0 + HID] = np.eye(HID, dtype=np.float32)
    for hh in range(H):
        wpack[hh * D:(hh + 1) * D, MSK0 + hh] = 1.0
    wpack[0, WL0] = W_lin[0, 0]
    wpack[0, WL0 + 1] = W_lin[1, 0]
    return wpack


def kernel(h, W_q, W_kv, W_mhc, W_lin, _trace=False):
    h = np.ascontiguousarray(np.asarray(h, dtype=np.float32))
    W_q = np.asarray(W_q, dtype=np.float32)
    W_kv = np.asarray(W_kv, dtype=np.float32)
    W_mhc = np.asarray(W_mhc, dtype=np.float32)
    W_lin = np.asarray(W_lin, dtype=np.float32)

    nc = _get_nc()
    wpack0 = _make_wpack(W_q, W_kv, W_mhc, W_lin)

    in_maps = []
    for core in range(8):
        b, half = core // 2, core % 2
        hn = h[b, :N, :]
        wp = wpack0.copy()
        wp[:, HG0] = h[b, N, :]
        # hnp[p, jc*128 + c] = hn[jc*128 + p, c]
        hnp = np.ascontiguousarray(
            hn.reshape(NJC, 128, HID).transpose(1, 0, 2).reshape(128, NJC * HID)
        )
        in_maps.append(
            {
                "wpack": wp,
                "hnT": np.ascontiguousarray(hn.T),
                "hnp": hnp,
                "hrT": np.ascontiguousarray(hn[half * ROWS:(half + 1) * ROWS, :].T),
            }
        )

    import time as _time

    kw = {}
    if _trace:
        import os

        kw = {"tmpdir": "/tmp/ktrace_" + str(os.getpid())}
        os.makedirs(kw["tmpdir"], exist_ok=True)
        print("[kernel] trace dir:", kw["tmpdir"], flush=True)
    _t = _time.time()
    print("[kernel] launching run_bass_kernel_spmd", flush=True)
    res = run_bass_kernel_spmd(nc, in_maps, core_ids=list(range(8)), trace=_trace, **kw)
    print(f"[kernel] run_bass_kernel_spmd done in {_time.time()-_t:.1f}s", flush=True)

    out = np.empty((BP, N * N, 1), dtype=np.float32)
    for core in range(8):
        b, half = core // 2, core % 2
        blk = res.results[core]["out"]  # (2048, 4096)
        out[b, half * ROWS * N:(half + 1) * ROWS * N, 0] = blk.ravel()
    if _trace:
        return out, res
    return out


# revision 15
# speedup vs baseline: 1.5473x; 1.5473x over previous
"""Trainium2 Bass kernel for nn_AttentionEdgeDecoder.

Reference computation (per batch b):
  hn = h[b,:4096,:], hg = h[b,4096,:]
  q = hg @ W_q  (single query, 8 heads x 16 dims)
  k,v = hn @ W_kv ; attn = softmax(q.k/sqrt(16)) ; y = attn.v
  mh = y @ W_mhc ; y2[i] = <mh, hn[i]>             (4096 scalars)
  e[i,j] = y2[j]*W_lin[0,0] + y2[i]*W_lin[1,0]     (4096x4096 output)

Output is 4*4096^2*4B = 268MB -> HBM-write bound. Sharding: 8 cores =
4 batches x 2 row-halves; each core computes y2[b] redundantly (tiny) and
streams its (2048, 4096) block of e to DRAM at the per-core HBM limit.

TensorEngine formulation (out = lhsT.T @ rhs):
  q_col   = matmul(lhsT=W_q, rhs=hg_col)                  [128,1]  f32
  Qh      = headmask * q_col   (block-diag scatter)       [128,8]  f32
  Wqeff   = matmul(lhsT=WkT, rhs=Qh) = Wk @ Qh            [128,8]  ->bf16
  sT      = matmul(lhsT=hnT_chunk, rhs=Wqeff)             [4096,8] bf16 mm
  pT      = exp(0.25*sT)      (no max-subtract: |s/4| < 8)         ->bf16
  u_raw   = sum_chunks matmul(lhsT=pT_chunk, rhs=hn_chunk)   [8,128]
  ssum    = matmul(lhsT=ones, rhs=pT) -> strided reduce      [1,8]
  u       = u_raw * (1/ssum)                                 [8,128] f32
  ymatT   = matmul(lhsT=Wv, rhs=u^T)                         [128,8] f32
  y_col   = reduce_h(ymatT * headmask)                       [128,1]
  mh_row  = matmul(lhsT=y_col, rhs=W_mhc)                    [1,128] f32
  mh0_rep = (W0*mh_row) x ones  (outer, K=1)                 [128,128]->bf16
  R       = matmul(lhsT=mh0_rep, rhs=hnT) = bcast W0*y2[j]   [128,4096]
  mh1_col = matmul(lhsT=mh_row, rhs=W1)  (K=1)               [128,1] ->bf16
  col     = matmul(lhsT=hrT_tile, rhs=mh1_col) = W1*y2[rows] [128,16]
  e_tile  = tensor_scalar_add(R, col[:,t])  (f32) -> 4MB DMAs out

Heavy matmuls run in bf16 (single PE pass + fast weight load; f32 is
two passes). Host ships hnT/hnp/hrT pre-cast to bf16 (halves input DMA).
A burst of dummy identity matmuls at t=0 warms the PE (HAM clock gate
4/8 -> 8/8) while input DMAs stream on both HWDGE rings.
"""

from contextlib import ExitStack

import ml_dtypes
import numpy as np

import concourse.bass as bass
import concourse.mybir as mybir
from concourse import bacc, tile
from concourse.bass_utils import run_bass_kernel_spmd

BP = 4
N = 4096
HID = 128
H = 8
D = 16
ROWS = N // 2          # 2048 rows per core
NT = ROWS // 128       # 16 row tiles per core
NJC = N // 128         # 32 node chunks
F32 = mybir.dt.float32
BF16 = mybir.dt.bfloat16

TPC = 2                # row-tiles per output chunk (4MB DMAs)
NCHUNK = NT // TPC
NWARM = 12             # PE warm-up matmuls (f32 -> 24 passes, ~5us cold)

# wpack column layout (all f32)
WKT0 = 0               # WkT = W_kv[:, :128].T
WV0 = HID              # Wv  = W_kv[:, 128:]
WMHC0 = 2 * HID
WQ0 = 3 * HID
ID0 = 4 * HID          # 128x128 identity
MSK0 = 5 * HID         # head mask [128, 8]
HG0 = 5 * HID + H      # hg column
WL0 = HG0 + 1          # W_lin row (partition 0)
WPACK_COLS = WL0 + 2


def build_bass():
    nc = bacc.Bacc()

    wpack_ext = nc.declare_dram_parameter("wpack", [HID, WPACK_COLS], F32, isOutput=False)
    hnT_ext = nc.declare_dram_parameter("hnT", [HID, N], BF16, isOutput=False)
    # hnp: hn pre-packed on host to [p, jc, c] = hn[jc*128+p, c], contiguous
    hnp_ext = nc.declare_dram_parameter("hnp", [128, NJC * HID], BF16, isOutput=False)
    hrT_ext = nc.declare_dram_parameter("hrT", [HID, ROWS], BF16, isOutput=False)
    out_ext = nc.declare_dram_parameter("out", [ROWS, N], F32, isOutput=True)

    with tile.TileContext(nc) as tc, ExitStack() as ctx:
        sb = ctx.enter_context(tc.tile_pool(name="sb", bufs=1))
        small = ctx.enter_context(tc.tile_pool(name="small", bufs=1))
        ps_acc = ctx.enter_context(tc.tile_pool(name="ps_acc", bufs=1, space="PSUM"))
        ps_tmp = ctx.enter_context(tc.tile_pool(name="ps_tmp", bufs=2, space="PSUM"))
        ps_big = ctx.enter_context(tc.tile_pool(name="ps_big", bufs=3, space="PSUM"))
        epool = ctx.enter_context(tc.tile_pool(name="epool", bufs=3))

        # ---- input DMAs: sync ring takes wpack+hnT, scalar ring hnp+hrT ----
        wpack_sb = sb.tile([HID, WPACK_COLS], F32)
        nc.sync.dma_start(wpack_sb[:], wpack_ext[:, :])
        hnT_sb = sb.tile([HID, N], BF16)
        for k in range(4):
            nc.sync.dma_start(
                hnT_sb[:, bass.ts(k, N // 4)], hnT_ext[:, bass.ts(k, N // 4)]
            )
        hn_sb = sb.tile([128, NJC, HID], BF16)
        hn_flat = hn_sb[:].rearrange("p a b -> p (a b)")
        for k in range(4):
            nc.scalar.dma_start(
                hn_flat[:, bass.ts(k, NJC * HID // 4)],
                hnp_ext[:, bass.ts(k, NJC * HID // 4)],
            )
        hrT_sb = sb.tile([HID, ROWS], BF16)
        nc.scalar.dma_start(hrT_sb[:], hrT_ext[:, :])

        identity = wpack_sb[:, ID0:ID0 + HID]
        mask_ap = wpack_sb[:, MSK0:MSK0 + H]

        # ---- PE warm-up: dense dependency-free matmuls right after wpack ----
        for w in range(NWARM):
            warm_ps = ps_big.tile([128, 128], F32, tag="big")
            nc.tensor.matmul(warm_ps[:], identity, identity, start=True, stop=True)

        ones_col = small.tile([128, 1], BF16)
        nc.vector.memset(ones_col[:], 1.0)
        ones_row = small.tile([1, 128], F32)
        nc.vector.memset(ones_row[:], 1.0)

        # ---- attention prologue ----
        # q_col = W_q.T @ hg
        q_ps = ps_tmp.tile([HID, 1], F32, tag="tmp")
        nc.tensor.matmul(
            q_ps[:], wpack_sb[:, WQ0:WQ0 + HID], wpack_sb[:, HG0:HG0 + 1],
            start=True, stop=True,
        )
        q_sb = small.tile([HID, 1], F32)
        nc.scalar.copy(q_sb[:], q_ps[:])

        # Qh block-diag scatter: Qh[e, h] = mask[e, h] * q[e]
        qh_sb = small.tile([HID, H], F32)
        nc.vector.tensor_scalar_mul(qh_sb[:], mask_ap, q_sb[:])

        # Wqeff = Wk @ Qh  (cast to bf16 on the PSUM->SBUF copy)
        wqeff_ps = ps_tmp.tile([HID, H], F32, tag="tmp")
        nc.tensor.matmul(
            wqeff_ps[:], wpack_sb[:, WKT0:WKT0 + HID], qh_sb[:], start=True, stop=True
        )
        wqeff_sb = small.tile([HID, H], BF16)
        nc.scalar.copy(wqeff_sb[:], wqeff_ps[:])

        # sT chunks: [j, h] scores, packed into one PSUM tensor [128, 32, 8]
        sT_ps = ps_acc.tile([128, NJC, H], F32)
        for jc in range(NJC):
            nc.tensor.matmul(
                sT_ps[:, jc, :],
                hnT_sb[:, bass.ts(jc, 128)],
                wqeff_sb[:],
                start=True,
                stop=True,
            )
        # pT = exp(0.25 * sT)   (one ACT op, bf16 out)
        pT_sb = small.tile([128, NJC, H], BF16)
        nc.scalar.activation(
            pT_sb[:], sT_ps[:], mybir.ActivationFunctionType.Exp, scale=0.25
        )

        # u_raw[h, c] accumulation over 32 chunks
        u_ps = ps_acc.tile([H, HID], F32)
        for jc in range(NJC):
            nc.tensor.matmul(
                u_ps[:],
                pT_sb[:, jc, :],
                hn_sb[:, jc, :],
                start=(jc == 0),
                stop=(jc == NJC - 1),
            )

        # ssum[h] = sum_j pT[j, h] via ones-matmul, then reduce over chunks
        sums_ps = ps_tmp.tile([1, NJC * H], F32, tag="tmp")
        nc.tensor.matmul(
            sums_ps[:],
            ones_col[:],
            pT_sb[:].rearrange("p a b -> p (a b)"),
            start=True,
            stop=True,
        )
        ssum_sb = small.tile([1, H], F32)
        nc.vector.tensor_reduce(
            ssum_sb[:],
            sums_ps[:].rearrange("p (a b) -> p b a", b=H),
            axis=mybir.AxisListType.X,
            op=mybir.AluOpType.add,
        )
        rr_sb = small.tile([1, H], F32)
        nc.vector.reciprocal(rr_sb[:], ssum_sb[:])
        rr_ps = ps_tmp.tile([H, 1], F32, tag="tmp")
        nc.tensor.transpose(rr_ps[:], rr_sb[:], identity[0:1, 0:1])
        rs_sb = small.tile([H, 1], F32)
        nc.scalar.copy(rs_sb[:], rr_ps[:])

        # u = u_raw / ssum  (per-partition scalar multiply)
        u_sb = small.tile([H, HID], F32)
        nc.vector.tensor_scalar_mul(u_sb[:], u_ps[:], rs_sb[:])

        # uT
        uT_ps = ps_tmp.tile([HID, H], F32, tag="tmp")
        nc.tensor.transpose(uT_ps[:], u_sb[:], identity[0:H, 0:H])
        uT_sb = small.tile([HID, H], F32)
        nc.scalar.copy(uT_sb[:], uT_ps[:])

        # ymatT = Wv.T @ uT  -> [e, h]
        ymatT_ps = ps_tmp.tile([HID, H], F32, tag="tmp")
        nc.tensor.matmul(
            ymatT_ps[:], wpack_sb[:, WV0:WV0 + HID], uT_sb[:], start=True, stop=True
        )
        # y_col[e] = ymatT[e, head(e)] = sum_h ymatT[e, h] * mask[e, h]
        ymm_sb = small.tile([HID, H], F32)
        y_sb = small.tile([HID, 1], F32)
        nc.vector.tensor_mul(ymm_sb[:], ymatT_ps[:], mask_ap)
        nc.vector.tensor_reduce(
            y_sb[:], ymm_sb[:], axis=mybir.AxisListType.X, op=mybir.AluOpType.add
        )

        # mh_row = y.T @ W_mhc
        mh_ps = ps_tmp.tile([1, HID], F32, tag="tmp")
        nc.tensor.matmul(
            mh_ps[:], y_sb[:], wpack_sb[:, WMHC0:WMHC0 + HID], start=True, stop=True
        )
        mh_sb = small.tile([1, HID], F32)
        nc.scalar.copy(mh_sb[:], mh_ps[:])

        # mh0_row = W0 * mh_row ; mh0_rep[c, p] = mh0[c] (bf16, 128 cols)
        mh0_sb = small.tile([1, HID], F32)
        nc.vector.tensor_scalar_mul(mh0_sb[:], mh_sb[:], wpack_sb[0:1, WL0:WL0 + 1])
        mh0rep_ps = ps_tmp.tile([HID, HID], F32, tag="tmp")
        nc.tensor.matmul(mh0rep_ps[:], mh0_sb[:], ones_row[:], start=True, stop=True)
        mh0rep_sb = small.tile([HID, HID], BF16)
        nc.scalar.copy(mh0rep_sb[:], mh0rep_ps[:])

        # mh1_col[c] = mh[c] * W1  (K=1 outer product with scalar, bf16)
        mh1_ps = ps_tmp.tile([HID, 1], F32, tag="tmp")
        nc.tensor.matmul(
            mh1_ps[:], mh_sb[:], wpack_sb[0:1, WL0 + 1:WL0 + 2], start=True, stop=True
        )
        mh1_sb = small.tile([HID, 1], BF16)
        nc.scalar.copy(mh1_sb[:], mh1_ps[:])

        # R[p, j] = W0*y2[j]: lhsT = mh0_rep (stationary), rhs = hnT chunks
        r_sb = sb.tile([128, N], F32)
        for k in range(8):
            rb_ps = ps_big.tile([128, 512], F32, tag="big")
            nc.tensor.matmul(
                rb_ps[:], mh0rep_sb[:], hnT_sb[:, bass.ts(k, 512)],
                start=True, stop=True,
            )
            if k % 2 == 0:
                nc.vector.tensor_copy(r_sb[:, bass.ts(k, 512)], rb_ps[:])
            else:
                nc.scalar.copy(r_sb[:, bass.ts(k, 512)], rb_ps[:])

        # col[p, t] = W1*y2[r0 + t*128 + p]
        col_ps = ps_acc.tile([128, NT], F32)
        for t in range(NT):
            nc.tensor.matmul(
                col_ps[:, t : t + 1],
                hrT_sb[:, bass.ts(t, 128)],
                mh1_sb[:],
                start=True,
                stop=True,
            )
        col_sb = small.tile([128, NT], F32)
        nc.vector.tensor_copy(col_sb[:], col_ps[:])

        # ---- epilogue: e tiles + DMA out ----
        out_r = out_ext[:, :].rearrange("(o s p) j -> o p s j", s=TPC, p=128)
        for ot in range(NCHUNK):
            etile = epool.tile([128, TPC, N], F32)
            for s in range(TPC):
                t = ot * TPC + s
                nc.vector.tensor_scalar_add(
                    etile[:, s, :], r_sb[:], col_sb[:, t : t + 1]
                )
            nc.sync.dma_start(out_r[ot], etile[:])

    nc.finalize()
    return nc


_CACHED = {}


def _get_nc():
    if "nc" not in _CACHED:
        _CACHED["nc"] = build_bass()
    return _CACHED["nc"]


def _make_wpack(W_q, W_kv, W_mhc, W_lin):
    wpack = np.zeros((HID, WPACK_COLS), dtype=np.float32)
    wpack[:, WKT0:WKT0 + HID] = W_kv[:, :HID].T
    wpack[:, WV0:WV0 + HID] = W_kv[:, HID:]
    wpack[:, WMHC0:WMHC0 + HID] = W_mhc
    wpack[:, WQ0:WQ0 + HID] = W_q
    wpack[:, ID0:ID0 + HID] = np.eye(HID, dtype=np.float32)
    for hh in range(H):
        wpack[hh * D:(hh + 1) * D, MSK0 + hh] = 1.0
    wpack[0, WL0] = W_lin[0, 0]
    wpack[0, WL0 + 1] = W_lin[1, 0]
    return wpack


def kernel(h, W_q, W_kv, W_mhc, W_lin, _trace=False):
    h = np.ascontiguousarray(np.asarray(h, dtype=np.float32))
    W_q = np.asarray(W_q, dtype=np.float32)
    W_kv = np.asarray(W_kv, dtype=np.float32)
    W_mhc = np.asarray(W_mhc, dtype=np.float32)
    W_lin = np.asarray(W_lin, dtype=np.float32)

    nc = _get_nc()
    wpack0 = _make_wpack(W_q, W_kv, W_mhc, W_lin)

    in_maps = []
    for core in range(8):
        b, half = core // 2, core % 2
        hn = h[b, :N, :]
        wp = wpack0.copy()
        wp[:, HG0] = h[b, N, :]
        hnb = hn.astype(ml_dtypes.bfloat16)
        # hnp[p, jc*128 + c] = hn[jc*128 + p, c]
        hnp = np.ascontiguousarray(
            hnb.reshape(NJC, 128, HID).transpose(1, 0, 2).reshape(128, NJC * HID)
        )
        in_maps.append(
            {
                "wpack": wp,
                "hnT": np.ascontiguousarray(hnb.T),
                "hnp": hnp,
                "hrT": np.ascontiguousarray(hnb[half * ROWS:(half + 1) * ROWS, :].T),
            }
        )

    import time as _time

    kw = {}
    if _trace:
        import os

        kw = {"tmpdir": "/tmp/ktrace_" + str(os.getpid())}
        os.makedirs(kw["tmpdir"], exist_ok=True)
        print("[kernel] trace dir:", kw["tmpdir"], flush=True)
    _t = _time.time()
    print("[kernel] launching run_bass_kernel_spmd", flush=True)
    res = run_bass_kernel_spmd(nc, in_maps, core_ids=list(range(8)), trace=_trace, **kw)
    print(f"[kernel] run_bass_kernel_spmd done in {_time.time()-_t:.1f}s", flush=True)

    out = np.empty((BP, N * N, 1), dtype=np.float32)
    for core in range(8):
        b, half = core // 2, core % 2
        blk = res.results[core]["out"]  # (2048, 4096)
        out[b, half * ROWS * N:(half + 1) * ROWS * N, 0] = blk.ravel()
    if _trace:
        return out, res
    return out
